# revision 41
# baseline (speedup 1.0000x reference)
"""MultiHeadGraphAttention Trainium2 kernel (v10, 357.5us measured).

v10 over v9: V projections are staged through qc0's cps PSUM half-bank
slots (garbage until kb0's start=True PV overwrites them) and emitted
AFTER the first score pair -- the V-MMs stream back-to-back instead of
serializing 16 MM->copy turns through the spool buffers, removing a
~10us ramp stall before the first exp (first ACTIVATE 33us -> 23us).

Data-parallel over batch: core b computes batch element b (B=8, 8 cores).

Per-core math (one batch element, N=2048 nodes, U=256 units, H=8 heads, d=32):
  Q = x Wq, K = x Wk, V = x Wv
  sT[k,q]  = sum_d KT[d,k] QT[d,q]           (scores, transposed layout)
  e        = exp(sT/sqrt(d)) * adjT          (masked exp)
  ctxT[d,q] = sum_k V[k,d] e[k,q] ; Z[q] = sum_k e[k,q]
  out      = (ctxT/Z).T @ Wo + bo

Structure (see trace-measured rationale below):
  - (qc, kb) blocks of [128 keys x 512 queries]; per block 4 head-PAIR
    PSUM score tiles [128, 2x512] (2 banks, double-buffered) pipeline
    ACT exp / DVE mask / PE matmuls across pairs and iterations.
  - Z is FUSED into the PV matmul: stationary [ones | V_h | zeros31]
    (M=64) makes row 0 of each 64-row block the softmax denominator.
  - Normalize: reciprocal over the whole cps tile (only Z rows 0/64 are
    read; junk rows are 0 -> Inf, never read), [1->32]-row broadcast
    DMAs into zinv, then one ctxn multiply per tile (frees cps).
  - Mask blocks kb0-2 jump ahead of wo/bo on the sync DMA ring.
  - wo4: Wo rows permuted into per-pair-tile chunks matching the fused
    ctx layout; host pre-transposes x/adj and converts to bf16.

Steady state (NTFF-measured) is bound jointly by the ACT exp rate
(4 x 1114ns ACTIVATEs per 4-pair group) and the PSUM spool recycle
chain (consumer + score MM + semaphore handoffs, 2 turns per 2-bank
buffer per group ~= 4.5us); both sit at ~4.4-4.5us/group at the
HAM-throttled 1.2GHz (K=4/8 -- the PE only un-throttles during the
dense projection prologue). Variants measured SLOWER and reverted:
  - DVE fused Schraudolph+mask (scalar_tensor_tensor, i16 bias tensor,
    1223ns/pair, numerically fine at rel err 7.8e-3): relieves ACT but
    lengthens the spool chain (1223 > 1114ns consumer) -> 368us.
  - GPSIMD mask multiplies and/or SWDGE mask DMAs: any SWDGE use makes
    the whole NEFF run at 1.0GHz instead of 1.2GHz -> 446us.
  - Merged 2-pair masks + global PV deferral: mask waits both exps,
    couples the two spool buffers -> 390us.
"""

import sys

for p in ("/opt/trn_rl_repo",):
    if p not in sys.path:
        sys.path.insert(0, p)

from contextlib import ExitStack

import numpy as np
import ml_dtypes

import concourse.bass as bass
import concourse.mybir as mybir
import concourse.tile as tile
from concourse import bacc
from concourse.bass_utils import run_bass_kernel_spmd

B, N, U, H, D = 8, 2048, 256, 8, 32
NB = N // 128
QC = 4
QW = N // QC
SCALE = 1.0 / np.sqrt(np.float32(D))
SCH_A = 128.0 * float(SCALE) * 1.4426950408889634
SCH_B = (127.0 - 0.04368) * 128.0

f32 = mybir.dt.float32
bf16 = mybir.dt.bfloat16
i16 = mybir.dt.int16
EXP = mybir.ActivationFunctionType.Exp
MULT = mybir.AluOpType.mult
ADD = mybir.AluOpType.add


def pair_mode(qc, kb, p):
    return "A"


def build_program():
    nc = bacc.Bacc("TRN2", target_bir_lowering=False, debug=False,
                   enable_asserts=False, num_devices=B)

    xT_d = nc.dram_tensor("xT", [U, N], bf16, kind="ExternalInput").ap()
    adjT_d = nc.dram_tensor("adjT", [N, N], bf16, kind="ExternalInput").ap()
    wq_d = nc.dram_tensor("Wq", [U, U], bf16, kind="ExternalInput").ap()
    wk_d = nc.dram_tensor("Wk", [U, U], bf16, kind="ExternalInput").ap()
    wv_d = nc.dram_tensor("Wv", [U, U], bf16, kind="ExternalInput").ap()
    wo4_d = nc.dram_tensor("wo4", [4 * 128, U], bf16, kind="ExternalInput").ap()
    bo_d = nc.dram_tensor("bo", [U], f32, kind="ExternalInput").ap()
    out_d = nc.dram_tensor("out", [N, U], f32, kind="ExternalOutput").ap()

    with tile.TileContext(nc) as tc:
        with ExitStack() as ctx:
            kernel_body(ctx, tc, xT_d, adjT_d, wq_d, wk_d, wv_d, wo4_d,
                        bo_d, out_d)
    nc.compile()
    return nc


def kernel_body(ctx, tc, xT_d, adjT_d, wq_d, wk_d, wv_d, wo4_d, bo_d, out_d):
    nc = tc.nc
    persist = ctx.enter_context(tc.tile_pool(name="persist", bufs=1))
    stage = ctx.enter_context(tc.tile_pool(name="stage", bufs=2))
    epool = ctx.enter_context(tc.tile_pool(name="epool", bufs=10))
    zpool = ctx.enter_context(tc.tile_pool(name="zpool", bufs=2))
    spool = ctx.enter_context(tc.tile_pool(name="spool", bufs=2, space="PSUM"))
    cpool = ctx.enter_context(tc.tile_pool(name="cpool", bufs=4, space="PSUM"))

    xT = [stage.tile([128, N], bf16, tag="stage", name=f"xT{c}") for c in range(2)]
    for c in range(2):
        nc.sync.dma_start(xT[c][:], xT_d[c * 128:(c + 1) * 128, :])
    w_sb = {}
    for nm, dram in (("wq", wq_d), ("wk", wk_d), ("wv", wv_d)):
        w_sb[nm] = persist.tile([128, 2 * U], bf16, tag=nm, name=nm)
        for c in range(2):
            nc.sync.dma_start(w_sb[nm][:, c * U:(c + 1) * U],
                              dram[c * 128:(c + 1) * 128, :])
    # first three mask blocks jump ahead of wo/bo on the sync ring (wo/bo
    # aren't read until the first out-proj ~40us in; kb0-2 exps are)
    m_sb = persist.tile([128, NB * N], bf16, tag="m")
    for kb in range(3):
        nc.sync.dma_start(m_sb[:, kb * N:(kb + 1) * N],
                          adjT_d[kb * 128:(kb + 1) * 128, :])
    wo_sb = persist.tile([128, 4 * U], bf16, tag="wo4")
    for t in range(4):
        nc.sync.dma_start(wo_sb[:, t * U:(t + 1) * U],
                          wo4_d[t * 128:(t + 1) * 128, :])
    bo_sb = persist.tile([1, U], f32, tag="bo")
    nc.sync.dma_start(bo_sb[:], bo_d.rearrange("(o n) -> o n", o=1))
    bo_bc = persist.tile([128, U], f32, tag="bo_bc")
    nc.sync.dma_start(bo_bc[:],
                      bo_sb[:].unsqueeze(1).broadcast_to([1, 128, U]))

    qT = [persist.tile([128, N], bf16, tag=f"qT{c}", name=f"qT{c}") for c in range(2)]
    kT = [persist.tile([128, N], bf16, tag=f"kT{c}", name=f"kT{c}") for c in range(2)]
    v_aug = persist.tile([128, NB * H * 64], bf16, tag="vaug")
    nc.vector.memset(v_aug[:], 0.0)
    nc.vector.memset(
        v_aug.rearrange("p (b c) -> p b c", c=64)[:, :, 0:1], 1.0)
    ctxn = [persist.tile([128, N], bf16, tag=f"ctxn{t}", name=f"ctxn{t}")
            for t in range(4)]
    out_sb = persist.tile([128, NB * U], f32, tag="out_sb")
    # 1/Z broadcast target: rows 1-32 / 65-96 per tile column-range get
    # the reciprocal; all other rows stay 1.0 forever (junk cps rows are
    # 0.0, and 0*finite=0 keeps ctxn junk rows NaN-free for wo4 zeros)
    zinv4 = persist.tile([128, 4 * QW], f32, tag="zinv4")
    nc.vector.memset(zinv4[:], 1.0)

    for nn in range(2):
        for g in range(2):
            for w, dst in (("wq", qT), ("wk", kT)):
                ps = spool.tile([128, 2 * QW], f32, tag="s", name="projps")
                for half in range(2):
                    sl = slice(half * QW, (half + 1) * QW)
                    tok = slice(nn * 2 * QW + half * QW,
                                nn * 2 * QW + (half + 1) * QW)
                    for kc in range(2):
                        nc.tensor.matmul(
                            ps[:, sl],
                            w_sb[w][:, (kc * 2 + g) * 128:(kc * 2 + g + 1) * 128],
                            xT[kc][:, tok],
                            start=(kc == 0), stop=(kc == 1))
                if w == "wq":
                    nc.scalar.copy(dst[g][:, nn * 2 * QW:(nn + 1) * 2 * QW], ps[:])
                else:
                    nc.vector.tensor_copy(dst[g][:, nn * 2 * QW:(nn + 1) * 2 * QW], ps[:])
    for kb in range(3, NB):
        nc.sync.dma_start(m_sb[:, kb * N:(kb + 1) * N],
                          adjT_d[kb * 128:(kb + 1) * 128, :])

    cps = {}

    def emit_vproj(kb):
        # V projection staged through a HALF of a qc0 cps tile: those
        # banks hold garbage until kb0's start=True PV overwrites them,
        # giving 8 free half-bank slots. V-MMs then stream back-to-back
        # (no spool contention, no MM->copy->MM serial chain), which
        # collapses the old ~10us V-phase ramp stall before the first exp.
        tile, half = (kb % 8) // 2, kb % 2
        dst = cps[0, tile][:, half * U:(half + 1) * U]
        for kc in range(2):
            nc.tensor.matmul(
                dst,
                xT[kc][:, kb * 128:(kb + 1) * 128],
                w_sb["wv"][:, kc * U:(kc + 1) * U],
                start=(kc == 0), stop=(kc == 1))
        nc.vector.tensor_copy(
            v_aug.rearrange("p (b c) -> p b c", c=64)
            [:, kb * H:(kb + 1) * H, 1:1 + D],
            dst.rearrange("p (h d) -> p h d", d=D))

    def emit_scores_pair(qc, kb, p):
        qs = qc * QW
        g, pi = p // 2, p % 2
        sps = spool.tile([128, 2 * QW], f32, tag="s", name=f"sps{qc}_{kb}_{p}")
        for jj in range(2):
            j = pi + 2 * jj
            nc.tensor.matmul(
                sps[:, jj * QW:(jj + 1) * QW],
                kT[g][32 * j:32 * (j + 1), kb * 128:(kb + 1) * 128],
                qT[g][32 * j:32 * (j + 1), qs:qs + QW],
                start=True, stop=True,
                tile_position=(32 * j, 0))
        return sps

    def emit_expmask_pv_pair(qc, kb, p, sps):
        qs = qc * QW
        g, pi = p // 2, p % 2
        e = epool.tile([128, 2 * QW], bf16, tag="e", name=f"e{qc}_{kb}_{p}")
        me = m_sb[:, kb * N + qs:kb * N + qs + QW]
        nc.scalar.activation(e[:], sps[:], EXP, scale=float(SCALE))
        nc.vector.tensor_tensor(
            e.rearrange("p (j q) -> p j q", j=2),
            e.rearrange("p (j q) -> p j q", j=2),
            me.unsqueeze(1).broadcast_to([128, 2, QW]), MULT)
        for jj in range(2):
            h = 4 * g + pi + 2 * jj
            ej = e[:, jj * QW:(jj + 1) * QW]
            nc.tensor.matmul(
                cps[qc, p][64 * jj:64 * jj + 64, :],
                v_aug[:, (kb * H + h) * 64:(kb * H + h + 1) * 64],
                ej, start=(kb == 0), stop=(kb == NB - 1),
                tile_position=(0, 64 * jj))

    def emit_zext(qc, t, tail=False):
        # reciprocal over the whole cps tile (DVE time is free-dim-bound;
        # junk rows give 1/0=Inf but only Z rows 0/64 are ever read),
        # then [1->32]-row broadcast DMAs into zinv. In the tail (after
        # the last exp) ACT is idle, so its HWDGE ring is free -- route
        # those broadcasts there so they don't FIFO behind the output
        # DMAs on the sync ring.
        zf = zpool.tile([128, QW], f32, tag="zf", name=f"zf{qc}_{t}")
        nc.vector.reciprocal_approx_fast(zf[:], cps[qc, t][:])
        dma = nc.scalar.dma_start if tail else nc.sync.dma_start
        for jj in range(2):
            dma(zinv4[64 * jj + 1:64 * jj + 33, t * QW:(t + 1) * QW],
                zf[64 * jj:64 * jj + 1, :]
                .unsqueeze(1).broadcast_to([1, 32, QW]))

    def emit_normalize_qc(qc):
        qs = qc * QW
        for t in range(4):
            nc.vector.tensor_tensor(ctxn[t][:, qs:qs + QW], cps[qc, t][:],
                                    zinv4[:, t * QW:(t + 1) * QW], MULT)

    def emit_outproj_qb(qb):
        ops = spool.tile([128, U], f32, tag="s", name=f"ops{qb}")
        for t in range(4):
            nc.tensor.matmul(
                ops[:],
                ctxn[t][:, qb * 128:(qb + 1) * 128],
                wo_sb[:, t * U:(t + 1) * U],
                start=(t == 0), stop=(t == 3))
        nc.vector.tensor_tensor(out_sb[:, qb * U:(qb + 1) * U], ops[:],
                                bo_bc[:], ADD)
        # the four TAIL output DMAs (qb 12-15, emitted after the last
        # exp) alternate between the two HWDGE rings: 4x128KB FIFO'd on
        # one ring was ~10us of pure drain; two rings halve it. ACT is
        # idle there, so its ring issue costs nothing.
        dma = nc.scalar.dma_start if qb in (13, 15) else nc.sync.dma_start
        dma(out_d[qb * 128:(qb + 1) * 128, :],
            out_sb[:, qb * U:(qb + 1) * U])

    prev = None
    pending_outproj = []
    first = True
    for qc in range(QC):
        for p in range(4):
            cps[qc, p] = cpool.tile([128, QW], f32, tag="c", name=f"cps{qc}_{p}")
        for kb in range(NB):
            for t in range(2):
                sp0 = emit_scores_pair(qc, kb, 2 * t)
                sp1 = emit_scores_pair(qc, kb, 2 * t + 1)
                if first:
                    # the V block rides the PE queue AFTER the first
                    # scores so the first exp isn't pushed behind it
                    for vkb in range(NB):
                        emit_vproj(vkb)
                    first = False
                if prev is not None:
                    (pqc, pkb, pt, psp0, psp1) = prev
                    emit_expmask_pv_pair(pqc, pkb, 2 * pt, psp0)
                    emit_expmask_pv_pair(pqc, pkb, 2 * pt + 1, psp1)
                    if pkb == NB - 1:
                        emit_zext(pqc, 2 * pt)
                        emit_zext(pqc, 2 * pt + 1)
                    if pkb == NB - 1 and pt == 1:
                        emit_normalize_qc(pqc)
                        pending_outproj += [pqc * QC + i for i in range(QC)]
                    elif (pending_outproj and pt == 1
                          and pkb in (1, 5, 9, 13)):
                        emit_outproj_qb(pending_outproj.pop(0))
                prev = (qc, kb, t, sp0, sp1)
    (pqc, pkb, pt, psp0, psp1) = prev
    emit_expmask_pv_pair(pqc, pkb, 2 * pt, psp0)
    emit_expmask_pv_pair(pqc, pkb, 2 * pt + 1, psp1)
    emit_zext(pqc, 2 * pt)
    emit_zext(pqc, 2 * pt + 1)
    emit_normalize_qc(pqc)
    pending_outproj += [pqc * QC + i for i in range(QC)]
    for qb in pending_outproj:
        emit_outproj_qb(qb)


_CACHED = None


def _get_program():
    global _CACHED
    if _CACHED is None:
        _CACHED = build_program()
    return _CACHED


def _bf16(a):
    return np.asarray(a, dtype=ml_dtypes.bfloat16)


def _build_wo4(Wo):
    wo4 = np.zeros((4 * 128, U), dtype=np.float32)
    Wo = np.asarray(Wo, np.float32)
    for g in range(2):
        for pi in range(2):
            t = 2 * g + pi
            for jj in range(2):
                h = 4 * g + pi + 2 * jj
                wo4[t * 128 + 64 * jj + 1:t * 128 + 64 * jj + 33, :] = \
                    Wo[h * D:(h + 1) * D, :]
    return _bf16(wo4)


def kernel(node_features, adjacency_matrix, Wq, Wk, Wv, Wo, bo, **run_kwargs):
    nc = _get_program()
    xT = _bf16(np.transpose(np.asarray(node_features, np.float32), (0, 2, 1)))
    adjT = _bf16(np.transpose(np.asarray(adjacency_matrix), (0, 2, 1)))
    wo4 = _build_wo4(Wo)
    wq, wk, wv = _bf16(Wq), _bf16(Wk), _bf16(Wv)
    bo32 = np.asarray(bo, np.float32)
    in_maps = []
    for b in range(B):
        in_maps.append({
            "xT": np.ascontiguousarray(xT[b]),
            "adjT": np.ascontiguousarray(adjT[b]),
            "Wq": wq, "Wk": wk, "Wv": wv, "wo4": wo4,
            "bo": bo32,
        })
    res = run_bass_kernel_spmd(nc, in_maps, core_ids=list(range(B)), **run_kwargs)
    out = np.stack([res.results[b]["out"] for b in range(B)], axis=0)
    kernel.last_results = res
    return out


# revision 42
# speedup vs baseline: 1.0063x; 1.0063x over previous
"""MultiHeadGraphAttention Trainium2 kernel (v10, 357.5us measured).

v10 over v9: V projections are staged through qc0's cps PSUM half-bank
slots (garbage until kb0's start=True PV overwrites them) and emitted
AFTER the first score pair -- the V-MMs stream back-to-back instead of
serializing 16 MM->copy turns through the spool buffers, removing a
~10us ramp stall before the first exp (first ACTIVATE 33us -> 23us).

Data-parallel over batch: core b computes batch element b (B=8, 8 cores).

Per-core math (one batch element, N=2048 nodes, U=256 units, H=8 heads, d=32):
  Q = x Wq, K = x Wk, V = x Wv
  sT[k,q]  = sum_d KT[d,k] QT[d,q]           (scores, transposed layout)
  e        = exp(sT/sqrt(d)) * adjT          (masked exp)
  ctxT[d,q] = sum_k V[k,d] e[k,q] ; Z[q] = sum_k e[k,q]
  out      = (ctxT/Z).T @ Wo + bo

Structure (see trace-measured rationale below):
  - (qc, kb) blocks of [128 keys x 512 queries]; per block 4 head-PAIR
    PSUM score tiles [128, 2x512] (2 banks, double-buffered) pipeline
    ACT exp / DVE mask / PE matmuls across pairs and iterations.
  - Z is FUSED into the PV matmul: stationary [ones | V_h | zeros31]
    (M=64) makes row 0 of each 64-row block the softmax denominator.
  - Normalize: reciprocal over the whole cps tile (only Z rows 0/64 are
    read; junk rows are 0 -> Inf, never read), [1->32]-row broadcast
    DMAs into zinv, then one ctxn multiply per tile (frees cps).
  - Mask blocks kb0-2 jump ahead of wo/bo on the sync DMA ring.
  - wo4: Wo rows permuted into per-pair-tile chunks matching the fused
    ctx layout; host pre-transposes x/adj and converts to bf16.

Steady state (NTFF-measured) is bound jointly by the ACT exp rate
(4 x 1114ns ACTIVATEs per 4-pair group) and the PSUM spool recycle
chain (consumer + score MM + semaphore handoffs, 2 turns per 2-bank
buffer per group ~= 4.5us); both sit at ~4.4-4.5us/group at the
HAM-throttled 1.2GHz (K=4/8 -- the PE only un-throttles during the
dense projection prologue). Variants measured SLOWER and reverted:
  - DVE fused Schraudolph+mask (scalar_tensor_tensor, i16 bias tensor,
    1223ns/pair, numerically fine at rel err 7.8e-3): relieves ACT but
    lengthens the spool chain (1223 > 1114ns consumer) -> 368us.
  - GPSIMD mask multiplies and/or SWDGE mask DMAs: any SWDGE use makes
    the whole NEFF run at 1.0GHz instead of 1.2GHz -> 446us.
  - Merged 2-pair masks + global PV deferral: mask waits both exps,
    couples the two spool buffers -> 390us.
"""

import sys

for p in ("/opt/trn_rl_repo",):
    if p not in sys.path:
        sys.path.insert(0, p)

from contextlib import ExitStack

import numpy as np
import ml_dtypes

import concourse.bass as bass
import concourse.mybir as mybir
import concourse.tile as tile
from concourse import bacc
from concourse.bass_utils import run_bass_kernel_spmd

B, N, U, H, D = 8, 2048, 256, 8, 32
NB = N // 128
QC = 4
QW = N // QC
SCALE = 1.0 / np.sqrt(np.float32(D))
SCH_A = 128.0 * float(SCALE) * 1.4426950408889634
SCH_B = (127.0 - 0.04368) * 128.0

f32 = mybir.dt.float32
bf16 = mybir.dt.bfloat16
i16 = mybir.dt.int16
EXP = mybir.ActivationFunctionType.Exp
MULT = mybir.AluOpType.mult
ADD = mybir.AluOpType.add


def pair_mode(qc, kb, p):
    return "A"


def build_program():
    nc = bacc.Bacc("TRN2", target_bir_lowering=False, debug=False,
                   enable_asserts=False, num_devices=B)

    xT_d = nc.dram_tensor("xT", [U, N], bf16, kind="ExternalInput").ap()
    adjT_d = nc.dram_tensor("adjT", [N, N], bf16, kind="ExternalInput").ap()
    wq_d = nc.dram_tensor("Wq", [U, U], bf16, kind="ExternalInput").ap()
    wk_d = nc.dram_tensor("Wk", [U, U], bf16, kind="ExternalInput").ap()
    wv_d = nc.dram_tensor("Wv", [U, U], bf16, kind="ExternalInput").ap()
    wo4_d = nc.dram_tensor("wo4", [4 * 128, U], bf16, kind="ExternalInput").ap()
    bo_d = nc.dram_tensor("bo", [U], f32, kind="ExternalInput").ap()
    out_d = nc.dram_tensor("out", [N, U], f32, kind="ExternalOutput").ap()

    with tile.TileContext(nc) as tc:
        with ExitStack() as ctx:
            kernel_body(ctx, tc, xT_d, adjT_d, wq_d, wk_d, wv_d, wo4_d,
                        bo_d, out_d)
    nc.compile()
    return nc


def kernel_body(ctx, tc, xT_d, adjT_d, wq_d, wk_d, wv_d, wo4_d, bo_d, out_d):
    nc = tc.nc
    persist = ctx.enter_context(tc.tile_pool(name="persist", bufs=1))
    stage = ctx.enter_context(tc.tile_pool(name="stage", bufs=2))
    epool = ctx.enter_context(tc.tile_pool(name="epool", bufs=10))
    zpool = ctx.enter_context(tc.tile_pool(name="zpool", bufs=2))
    spool = ctx.enter_context(tc.tile_pool(name="spool", bufs=2, space="PSUM"))
    cpool = ctx.enter_context(tc.tile_pool(name="cpool", bufs=4, space="PSUM"))

    xT = [stage.tile([128, N], bf16, tag="stage", name=f"xT{c}") for c in range(2)]
    for c in range(2):
        nc.sync.dma_start(xT[c][:], xT_d[c * 128:(c + 1) * 128, :])
    w_sb = {}
    for nm, dram in (("wq", wq_d), ("wk", wk_d), ("wv", wv_d)):
        w_sb[nm] = persist.tile([128, 2 * U], bf16, tag=nm, name=nm)
        for c in range(2):
            nc.sync.dma_start(w_sb[nm][:, c * U:(c + 1) * U],
                              dram[c * 128:(c + 1) * 128, :])
    # first three mask blocks jump ahead of wo/bo on the sync ring (wo/bo
    # aren't read until the first out-proj ~40us in; kb0-2 exps are)
    m_sb = persist.tile([128, NB * N], bf16, tag="m")
    for kb in range(3):
        nc.sync.dma_start(m_sb[:, kb * N:(kb + 1) * N],
                          adjT_d[kb * 128:(kb + 1) * 128, :])
    wo_sb = persist.tile([128, 4 * U], bf16, tag="wo4")
    for t in range(4):
        nc.sync.dma_start(wo_sb[:, t * U:(t + 1) * U],
                          wo4_d[t * 128:(t + 1) * 128, :])
    bo_sb = persist.tile([1, U], f32, tag="bo")
    nc.sync.dma_start(bo_sb[:], bo_d.rearrange("(o n) -> o n", o=1))
    bo_bc = persist.tile([128, U], f32, tag="bo_bc")
    nc.sync.dma_start(bo_bc[:],
                      bo_sb[:].unsqueeze(1).broadcast_to([1, 128, U]))

    qT = [persist.tile([128, N], bf16, tag=f"qT{c}", name=f"qT{c}") for c in range(2)]
    kT = [persist.tile([128, N], bf16, tag=f"kT{c}", name=f"kT{c}") for c in range(2)]
    v_aug = persist.tile([128, NB * H * 64], bf16, tag="vaug")
    nc.vector.memset(v_aug[:], 0.0)
    nc.vector.memset(
        v_aug.rearrange("p (b c) -> p b c", c=64)[:, :, 0:1], 1.0)
    ctxn = [persist.tile([128, N], bf16, tag=f"ctxn{t}", name=f"ctxn{t}")
            for t in range(4)]
    out_sb = persist.tile([128, NB * U], f32, tag="out_sb")
    # 1/Z broadcast target: rows 1-32 / 65-96 per tile column-range get
    # the reciprocal; all other rows stay 1.0 forever (junk cps rows are
    # 0.0, and 0*finite=0 keeps ctxn junk rows NaN-free for wo4 zeros)
    zinv4 = persist.tile([128, 4 * QW], f32, tag="zinv4")
    nc.vector.memset(zinv4[:], 1.0)

    for nn in range(2):
        for g in range(2):
            for w, dst in (("wq", qT), ("wk", kT)):
                ps = spool.tile([128, 2 * QW], f32, tag="s", name="projps")
                for half in range(2):
                    sl = slice(half * QW, (half + 1) * QW)
                    tok = slice(nn * 2 * QW + half * QW,
                                nn * 2 * QW + (half + 1) * QW)
                    for kc in range(2):
                        nc.tensor.matmul(
                            ps[:, sl],
                            w_sb[w][:, (kc * 2 + g) * 128:(kc * 2 + g + 1) * 128],
                            xT[kc][:, tok],
                            start=(kc == 0), stop=(kc == 1))
                if w == "wq":
                    nc.scalar.copy(dst[g][:, nn * 2 * QW:(nn + 1) * 2 * QW], ps[:])
                else:
                    nc.vector.tensor_copy(dst[g][:, nn * 2 * QW:(nn + 1) * 2 * QW], ps[:])
    for kb in range(3, NB):
        nc.sync.dma_start(m_sb[:, kb * N:(kb + 1) * N],
                          adjT_d[kb * 128:(kb + 1) * 128, :])

    cps = {}

    def emit_vproj(kb):
        # V projection staged through a HALF of a qc0 cps tile: those
        # banks hold garbage until kb0's start=True PV overwrites them,
        # giving 8 free half-bank slots. V-MMs then stream back-to-back
        # (no spool contention, no MM->copy->MM serial chain), which
        # collapses the old ~10us V-phase ramp stall before the first exp.
        tile, half = (kb % 8) // 2, kb % 2
        dst = cps[0, tile][:, half * U:(half + 1) * U]
        for kc in range(2):
            nc.tensor.matmul(
                dst,
                xT[kc][:, kb * 128:(kb + 1) * 128],
                w_sb["wv"][:, kc * U:(kc + 1) * U],
                start=(kc == 0), stop=(kc == 1))
        nc.vector.tensor_copy(
            v_aug.rearrange("p (b c) -> p b c", c=64)
            [:, kb * H:(kb + 1) * H, 1:1 + D],
            dst.rearrange("p (h d) -> p h d", d=D))

    def emit_scores_pair(qc, kb, p):
        qs = qc * QW
        g, pi = p // 2, p % 2
        sps = spool.tile([128, 2 * QW], f32, tag="s", name=f"sps{qc}_{kb}_{p}")
        for jj in range(2):
            j = pi + 2 * jj
            nc.tensor.matmul(
                sps[:, jj * QW:(jj + 1) * QW],
                kT[g][32 * j:32 * (j + 1), kb * 128:(kb + 1) * 128],
                qT[g][32 * j:32 * (j + 1), qs:qs + QW],
                start=True, stop=True,
                tile_position=(32 * j, 0))
        return sps

    def emit_expmask_pv_pair(qc, kb, p, sps):
        qs = qc * QW
        g, pi = p // 2, p % 2
        e = epool.tile([128, 2 * QW], bf16, tag="e", name=f"e{qc}_{kb}_{p}")
        me = m_sb[:, kb * N + qs:kb * N + qs + QW]
        nc.scalar.activation(e[:], sps[:], EXP, scale=float(SCALE))
        nc.vector.tensor_tensor(
            e.rearrange("p (j q) -> p j q", j=2),
            e.rearrange("p (j q) -> p j q", j=2),
            me.unsqueeze(1).broadcast_to([128, 2, QW]), MULT)
        for jj in range(2):
            h = 4 * g + pi + 2 * jj
            ej = e[:, jj * QW:(jj + 1) * QW]
            nc.tensor.matmul(
                cps[qc, p][64 * jj:64 * jj + 64, :],
                v_aug[:, (kb * H + h) * 64:(kb * H + h + 1) * 64],
                ej, start=(kb == 0), stop=(kb == NB - 1),
                tile_position=(0, 64 * jj))

    def emit_zext(qc, t, tail=False):
        # reciprocal over the whole cps tile (DVE time is free-dim-bound;
        # junk rows give 1/0=Inf but only Z rows 0/64 are ever read),
        # then [1->32]-row broadcast DMAs into zinv. In the tail (after
        # the last exp) ACT is idle, so its HWDGE ring is free -- route
        # those broadcasts there so they don't FIFO behind the output
        # DMAs on the sync ring.
        zf = zpool.tile([128, QW], f32, tag="zf", name=f"zf{qc}_{t}")
        nc.vector.reciprocal_approx_fast(zf[:], cps[qc, t][:])
        dma = nc.scalar.dma_start if tail else nc.sync.dma_start
        for jj in range(2):
            dma(zinv4[64 * jj + 1:64 * jj + 33, t * QW:(t + 1) * QW],
                zf[64 * jj:64 * jj + 1, :]
                .unsqueeze(1).broadcast_to([1, 32, QW]))

    def emit_normalize_qc(qc):
        qs = qc * QW
        for t in range(4):
            nc.vector.tensor_tensor(ctxn[t][:, qs:qs + QW], cps[qc, t][:],
                                    zinv4[:, t * QW:(t + 1) * QW], MULT)

    def emit_outproj_qb(qb):
        ops = spool.tile([128, U], f32, tag="s", name=f"ops{qb}")
        for t in range(4):
            nc.tensor.matmul(
                ops[:],
                ctxn[t][:, qb * 128:(qb + 1) * 128],
                wo_sb[:, t * U:(t + 1) * U],
                start=(t == 0), stop=(t == 3))
        nc.vector.tensor_tensor(out_sb[:, qb * U:(qb + 1) * U], ops[:],
                                bo_bc[:], ADD)
        # all output DMAs stay on the sync HWDGE ring: routing the tail
        # ones (or the tail zinv broadcasts) through the scalar ring was
        # measured SLOWER (359.2 / 361.8us vs 357.5us) -- the scalar
        # HWDGE path underperforms the sync one here
        nc.sync.dma_start(
            out_d[qb * 128:(qb + 1) * 128, :],
            out_sb[:, qb * U:(qb + 1) * U])

    prev = None
    pending_outproj = []
    first = True
    for qc in range(QC):
        for p in range(4):
            cps[qc, p] = cpool.tile([128, QW], f32, tag="c", name=f"cps{qc}_{p}")
        for kb in range(NB):
            for t in range(2):
                sp0 = emit_scores_pair(qc, kb, 2 * t)
                sp1 = emit_scores_pair(qc, kb, 2 * t + 1)
                if first:
                    # the V block rides the PE queue AFTER the first
                    # scores so the first exp isn't pushed behind it
                    for vkb in range(NB):
                        emit_vproj(vkb)
                    first = False
                if prev is not None:
                    (pqc, pkb, pt, psp0, psp1) = prev
                    emit_expmask_pv_pair(pqc, pkb, 2 * pt, psp0)
                    emit_expmask_pv_pair(pqc, pkb, 2 * pt + 1, psp1)
                    if pkb == NB - 1:
                        emit_zext(pqc, 2 * pt)
                        emit_zext(pqc, 2 * pt + 1)
                    if pkb == NB - 1 and pt == 1:
                        emit_normalize_qc(pqc)
                        pending_outproj += [pqc * QC + i for i in range(QC)]
                    elif (pending_outproj and pt == 1
                          and pkb in (1, 5, 9, 13)):
                        emit_outproj_qb(pending_outproj.pop(0))
                prev = (qc, kb, t, sp0, sp1)
    (pqc, pkb, pt, psp0, psp1) = prev
    emit_expmask_pv_pair(pqc, pkb, 2 * pt, psp0)
    emit_expmask_pv_pair(pqc, pkb, 2 * pt + 1, psp1)
    emit_zext(pqc, 2 * pt)
    emit_zext(pqc, 2 * pt + 1)
    emit_normalize_qc(pqc)
    pending_outproj += [pqc * QC + i for i in range(QC)]
    for qb in pending_outproj:
        emit_outproj_qb(qb)


_CACHED = None


def _get_program():
    global _CACHED
    if _CACHED is None:
        _CACHED = build_program()
    return _CACHED


def _bf16(a):
    return np.asarray(a, dtype=ml_dtypes.bfloat16)


def _build_wo4(Wo):
    wo4 = np.zeros((4 * 128, U), dtype=np.float32)
    Wo = np.asarray(Wo, np.float32)
    for g in range(2):
        for pi in range(2):
            t = 2 * g + pi
            for jj in range(2):
                h = 4 * g + pi + 2 * jj
                wo4[t * 128 + 64 * jj + 1:t * 128 + 64 * jj + 33, :] = \
                    Wo[h * D:(h + 1) * D, :]
    return _bf16(wo4)


def kernel(node_features, adjacency_matrix, Wq, Wk, Wv, Wo, bo, **run_kwargs):
    nc = _get_program()
    xT = _bf16(np.transpose(np.asarray(node_features, np.float32), (0, 2, 1)))
    adjT = _bf16(np.transpose(np.asarray(adjacency_matrix), (0, 2, 1)))
    wo4 = _build_wo4(Wo)
    wq, wk, wv = _bf16(Wq), _bf16(Wk), _bf16(Wv)
    bo32 = np.asarray(bo, np.float32)
    in_maps = []
    for b in range(B):
        in_maps.append({
            "xT": np.ascontiguousarray(xT[b]),
            "adjT": np.ascontiguousarray(adjT[b]),
            "Wq": wq, "Wk": wk, "Wv": wv, "wo4": wo4,
            "bo": bo32,
        })
    res = run_bass_kernel_spmd(nc, in_maps, core_ids=list(range(B)), **run_kwargs)
    out = np.stack([res.results[b]["out"] for b in range(B)], axis=0)
    kernel.last_results = res
    return out
